# revision 36
# baseline (speedup 1.0000x reference)
"""MHSA kernel for 8 Trainium2 NeuronCores (Bass/Tile).

Distribution: data-parallel over batch (4) x tensor-parallel over heads
(2 groups of 8 heads) = 8 shards, one per core.

Per core (batch b, head-group t), everything bf16 on the PE with fp32
PSUM accumulation:
  xT[c,n] (host-transposed)  -> qT,kT[d,n] d-major;  v[m,d] m-major with a
  ones column appended per head (so the attn@v matmul also produces the
  softmax denominator).  Softmax skips max-subtraction (scores ~ N(0,1));
  exp runs on ACT straight out of PSUM.  vhat is normalized from PSUM via
  a reciprocal + partition-broadcast DMA, and the output projection
  consumes vhatT directly, emitting the partial output in natural [n,c]
  layout.  Host sums the two TP partials per batch and adds the bias.
"""

import numpy as np
import ml_dtypes
import jax
from jax.sharding import Mesh, PartitionSpec as P

import concourse.bass as bass
import concourse.tile as tile
from concourse import mybir
from concourse.bass import ds, ts
from concourse.bass2jax import bass_jit, bass_shard_map

# ---- problem constants --------------------------------------------------
B, N, C, H = 4, 2048, 1024, 16
HD = C // H           # 64
NCORES = 8
TP = 2                # head groups
HPG = H // TP         # 8 heads per core
DPG = HPG * HD        # 512
VSTR = HD + 8         # v_aug per-head stride: 64 v dims + 1 ones + 7 pad
VAUG = HPG * VSTR     # 576
PP = 128              # partitions
SB = 512              # matmul free-dim block (one PSUM bank of fp32)

BF = mybir.dt.bfloat16
F32 = mybir.dt.float32
EXP = mybir.ActivationFunctionType.Exp
MULT = mybir.AluOpType.mult

_compiled = {}


# ---- the per-core Tile kernel ------------------------------------------
def _build_mhsa(tc, out_ap, xT, wq, wk, wv, wo, n=N, c=C, hpg=HPG, dbg=None,
                dbg2=None, exact_recip=True, ablate=(), repeat=1):
    """Per-core MHSA.  APs (no leading core dim):
      xT [c, n] bf16, wq/wk [c, dpg] bf16, wv [c, vaug] bf16,
      wo [HD, hpg, c] bf16, out [n, c] f32.
    """
    nc = tc.nc
    dpg = hpg * HD
    vaug = hpg * VSTR
    nct = c // PP          # c-tiles (contraction for projections)
    nmt = n // PP          # m-tiles (keys) == n-tiles (queries)
    ndt = dpg // PP        # d-tiles for q/k
    sbn = min(SB, n)       # free-dim block for projection outputs
    sbc = min(SB, c)
    nhalf = n // 2         # attention processes n in two halves
    sca = min(SB, nhalf)   # score sub-block
    nsc = nhalf // sca

    with tc.tile_pool(name="persist", bufs=1) as persist:
        xT_sb = persist.tile([PP, nct, n], BF)
        wq_sb = persist.tile([PP, nct, dpg], BF)
        wk_sb = persist.tile([PP, nct, dpg], BF)
        wv_sb = persist.tile([PP, nct, vaug], BF)
        wo_sb = persist.tile([PP, ndt, c], BF)
        qT_sb = persist.tile([PP, ndt, n], BF)
        kT_sb = persist.tile([PP, ndt, n], BF)
        v_sb = persist.tile([PP, nmt, vaug], BF)
        vhn_sb = persist.tile([PP, ndt, n], BF)   # unnormalized vhatT
        vhn2_sb = persist.tile([PP, ndt, n], BF)  # normalized vhatT

        xT_t = xT.rearrange("(t p) n -> t p n", p=PP)
        for c_t in range(nct):
            nc.sync.dma_start(out=xT_sb[:, c_t], in_=xT_t[c_t])
        nc.sync.dma_start(out=wq_sb, in_=wq.rearrange("(t p) d -> p t d", p=PP))
        nc.sync.dma_start(out=wk_sb, in_=wk.rearrange("(t p) d -> p t d", p=PP))
        nc.sync.dma_start(out=wv_sb, in_=wv.rearrange("(t p) d -> p t d", p=PP))
        nc.sync.dma_start(out=wo_sb, in_=wo.rearrange("(t p) c -> p t c", p=PP))

        # ---- phase 1: qT, kT = (w @ xT) in d-major [d, n] layout -------
        for _rep in range(repeat):
            _phases(tc, out_ap, xT, wq, wk, wv, wo, n, c, hpg, dbg, dbg2,
                    exact_recip, ablate, persist, xT_sb, wq_sb, wk_sb, wv_sb,
                    wo_sb, qT_sb, kT_sb, v_sb, vhn_sb, vhn2_sb)


def _phases(tc, out_ap, xT, wq, wk, wv, wo, n, c, hpg, dbg, dbg2, exact_recip,
            ablate, persist, xT_sb, wq_sb, wk_sb, wv_sb, wo_sb, qT_sb, kT_sb,
            v_sb, vhn_sb, vhn2_sb):
    nc = tc.nc
    dpg = hpg * HD
    vaug = hpg * VSTR
    nct = c // PP
    nmt = n // PP
    ndt = dpg // PP
    sbn = min(SB, n)
    sbc = min(SB, c)
    nhalf = n // 2
    sca = min(SB, nhalf)
    nsc = nhalf // sca
    if True:
        # d_t outer + q/k interleaved so attention pair 0 can start as soon
        # as v is ready; one weight tile serves all n-blocks (4 MMs per LDW)
        with tc.tile_pool(name="qk_ps", bufs=8, space=bass.MemorySpace.PSUM) as qkp:
            for d_t in range(ndt):
                for w_sb, dst in ((wk_sb, kT_sb), (wq_sb, qT_sb)):
                    pss = [qkp.tile([PP, sbn], F32, tag="qk", name="qk_ps_t")
                           for _ in range(n // sbn)]
                    for c_t in range(nct):
                        for nb in range(n // sbn):
                            nc.tensor.matmul(
                                pss[nb],
                                w_sb[:, c_t, ts(d_t, PP)],
                                xT_sb[:, c_t, ts(nb, sbn)],
                                start=(c_t == 0),
                                stop=(c_t == nct - 1),
                            )
                    for nb in range(n // sbn):
                        nc.vector.tensor_copy(dst[:, d_t, ts(nb, sbn)], pss[nb])

        # ---- phase 2: v = (x @ wv) m-major, aug layout + ones ----------
        with tc.tile_pool(name="v_ps", bufs=2, space=bass.MemorySpace.PSUM) as vp:
            for m_t in range(nmt):
                ps = vp.tile([PP, vaug], F32)
                for c_t in range(nct):
                    st, sp = (c_t == 0), (c_t == nct - 1)
                    for vb in range(0, vaug, SB):
                        ve = min(vb + SB, vaug)
                        nc.tensor.matmul(
                            ps[:, vb:ve],
                            xT_sb[:, c_t, ts(m_t, PP)],
                            wv_sb[:, c_t, vb:ve],
                            start=st,
                            stop=sp,
                        )
                nc.vector.tensor_copy(v_sb[:, m_t], ps)
                nc.vector.memset(
                    v_sb[:, m_t].rearrange("p (h w) -> p h w", w=VSTR)[:, :, HD : HD + 1],
                    1.0,
                )

        # ---- phase 3: attention, head PAIRS row-packed in the PE array --
        # Heads 2j (partitions 0:64) and 2j+1 (partitions 64:128) of kT/qT
        # tile j issue concurrent K=64 matmuls to different row groups and
        # different PSUM banks; one exp covers both heads' score block.
        with (
            tc.tile_pool(name="sc_ps", bufs=2, space=bass.MemorySpace.PSUM) as scp,
            tc.tile_pool(name="vh_ps", bufs=4, space=bass.MemorySpace.PSUM) as vhp,
            tc.tile_pool(name="e_sb", bufs=4) as ep,
            tc.tile_pool(name="bc_sb", bufs=4) as bcp,
            tc.tile_pool(name="den_sb", bufs=2) as denp,
            tc.tile_pool(name="rec_dr", bufs=2, space=bass.MemorySpace.DRAM) as drp,
        ):
            if "attn" in ablate:
                nc.vector.memset(vhn2_sb, 1.0)
            nun = 2 * (n // sca)              # den rows per head pair
            for j in range(0 if "attn" not in ablate else hpg // 2, hpg // 2):
                kA, qA = kT_sb[0:HD, j], qT_sb[0:HD, j]        # head 2j
                kB, qB = kT_sb[HD:PP, j], qT_sb[HD:PP, j]      # head 2j+1
                # den rows live at the 4 legal engine partition bases x 2
                # column halves (engine APs only address bases 0/32/64/96)
                den_p = denp.tile([97, 2 * sca], F32)
                nc.vector.memset(den_p, 1.0)
                for nb in range(n // sca):
                    vhA = vhp.tile([HD + 1, sca], F32, tag="vh")
                    vhB = vhp.tile([HD + 1, sca], F32, tag="vh")
                    for m_t in range(nmt):
                        sc = scp.tile([PP, 2 * sca], F32)
                        nc.tensor.matmul(
                            sc[:, 0:sca], kA[:, ts(m_t, PP)],
                            qA[:, ts(nb, sca)], start=True, stop=True,
                        )
                        nc.tensor.matmul(
                            sc[:, sca : 2 * sca], kB[:, ts(m_t, PP)],
                            qB[:, ts(nb, sca)], start=True, stop=True,
                        )
                        E = ep.tile([PP, 2 * sca], BF)
                        if "exp" in ablate:
                            nc.vector.tensor_copy(E, sc)
                        else:
                            nc.scalar.activation(E, sc, EXP)
                        st, sp = (m_t == 0), (m_t == nmt - 1)
                        nc.tensor.matmul(
                            vhA, v_sb[:, m_t, ds((2 * j) * VSTR, HD + 1)],
                            E[:, 0:sca], start=st, stop=sp,
                        )
                        nc.tensor.matmul(
                            vhB, v_sb[:, m_t, ds((2 * j + 1) * VSTR, HD + 1)],
                            E[:, sca : 2 * sca], start=st, stop=sp,
                        )
                    for vh, pb, u in ((vhA, 0, 2 * nb), (vhB, HD, 2 * nb + 1)):
                        # stash unnormalized vhat + its denominator row
                        nc.vector.tensor_copy(
                            vhn_sb[pb : pb + HD, j, ts(nb, sca)], vh[0:HD]
                        )
                        ub = 32 * (u % 4)
                        nc.vector.tensor_copy(
                            den_p[ub : ub + 1, ts(u // 4, sca)], vh[HD : HD + 1, :]
                        )
                if "norm" in ablate:
                    nc.vector.tensor_copy(vhn2_sb[:, j], vhn_sb[:, j])
                    continue
                # one batched reciprocal for the pair's denominators, then
                # partition-broadcast each row via a DRAM bounce
                rec_p = denp.tile([97, 2 * sca], F32, tag="rec")
                nc.vector.reciprocal(rec_p, den_p)
                dr = drp.tile([nun, sca], F32)
                for u in range(nun):
                    ub = 32 * (u % 4)
                    nc.sync.dma_start(
                        out=dr[u : u + 1],
                        in_=rec_p[ub : ub + 1, ts(u // 4, sca)],
                    )
                for nb in range(n // sca):
                    for pb, u in ((0, 2 * nb), (HD, 2 * nb + 1)):
                        bc = bcp.tile([PP, sca], F32)
                        row = dr[u : u + 1]
                        nc.sync.dma_start(
                            out=bc,
                            in_=bass.AP(
                                tensor=row.tensor, offset=row.offset,
                                ap=[[0, PP], [1, sca]],
                            ),
                        )
                        nc.vector.tensor_tensor(
                            vhn2_sb[pb : pb + HD, j, ts(nb, sca)],
                            vhn_sb[pb : pb + HD, j, ts(nb, sca)],
                            bc[pb : pb + HD], MULT,
                        )

        if dbg is not None:
            d_qT, d_kT, d_v, d_vhn = dbg
            nc.sync.dma_start(out=d_qT, in_=qT_sb)
            nc.sync.dma_start(out=d_kT, in_=kT_sb)
            nc.sync.dma_start(out=d_v, in_=v_sb)
            nc.sync.dma_start(out=d_vhn, in_=vhn2_sb)

        # ---- phase 4: out = vhatT.T @ woT in natural [n, c] layout -----
        with (
            tc.tile_pool(name="o_ps", bufs=4, space=bass.MemorySpace.PSUM) as op,
            tc.tile_pool(name="o_sb", bufs=4) as osb,
        ):
            for n_t in range(nmt):
                for cb in range(c // sbc):
                    ps = op.tile([PP, sbc], F32)
                    for d_t in range(ndt):
                        nc.tensor.matmul(
                            ps,
                            vhn2_sb[:, d_t, ts(n_t, PP)],
                            wo_sb[:, d_t, ts(cb, sbc)],
                            start=(d_t == 0),
                            stop=(d_t == ndt - 1),
                        )
                    stg = osb.tile([PP, sbc], F32)
                    nc.vector.tensor_copy(stg, ps)
                    nc.sync.dma_start(
                        out=out_ap[ts(n_t, PP), ts(cb, sbc)], in_=stg
                    )


@bass_jit
def _mhsa_core(nc, xT, wq, wk, wv, wo):
    out = nc.dram_tensor("out_part", [N, C], F32, kind="ExternalOutput")
    with tile.TileContext(nc) as tc:
        _build_mhsa(tc, out[:], xT[:], wq[:], wk[:], wv[:], wo[:])
    return (out,)


# ---- host-side sharding / driver ---------------------------------------
def _get_compiled():
    if "fn" in _compiled:
        return _compiled["fn"], _compiled["mesh"]
    devs = jax.devices()[:NCORES]
    mesh = Mesh(np.asarray(devs), ("core",))
    fn = bass_shard_map(
        _mhsa_core,
        mesh=mesh,
        in_specs=(P("core"),) * 5,
        out_specs=(P("core"),),
    )
    _compiled["fn"] = fn
    _compiled["mesh"] = mesh
    return fn, mesh


def _make_shards(x, w_qkv, w_out):
    bf = ml_dtypes.bfloat16
    wq_s = w_qkv[0:C] * np.float32(HD**-0.5)  # fold attention scale into wq
    wk_s = w_qkv[C : 2 * C]
    wv_s = w_qkv[2 * C : 3 * C]
    xT_b = [np.ascontiguousarray(x[b].T).astype(bf) for b in range(B)]
    xTs, wqs, wks, wvs, wos = [], [], [], [], []
    for cid in range(NCORES):
        b, t = cid // TP, cid % TP
        sl = slice(t * DPG, (t + 1) * DPG)
        xTs.append(xT_b[b])
        wqs.append(np.ascontiguousarray(wq_s[sl].T).astype(bf))
        wks.append(np.ascontiguousarray(wk_s[sl].T).astype(bf))
        wvT = wv_s[sl].T  # [C, DPG]
        wv_aug = np.zeros((C, VAUG), bf)
        for h in range(HPG):
            wv_aug[:, h * VSTR : h * VSTR + HD] = wvT[:, h * HD : (h + 1) * HD].astype(bf)
        wvs.append(wv_aug)
        woT = w_out[:, sl].T  # [DPG, C]
        wos.append(np.ascontiguousarray(woT).astype(bf))
    return tuple(
        np.concatenate(a, axis=0) for a in (xTs, wqs, wks, wvs, wos)
    )


def kernel(x, w_qkv, w_out, b_out):
    x = np.asarray(x, dtype=np.float32)
    w_qkv = np.asarray(w_qkv, dtype=np.float32)
    w_out = np.asarray(w_out, dtype=np.float32)
    b_out = np.asarray(b_out, dtype=np.float32)

    fn, _ = _get_compiled()
    shards = _make_shards(x, w_qkv, w_out)
    res = jax.block_until_ready(fn(*shards))
    parts = np.asarray(res[0]).reshape(NCORES, N, C)

    out = np.empty((B, N, C), dtype=np.float32)
    for b in range(B):
        out[b] = parts[2 * b] + parts[2 * b + 1] + b_out[None, :]
    return out


if __name__ == "__main__":
    rng = np.random.default_rng(0)
    x = rng.standard_normal((B, N, C), dtype=np.float32)
    w_qkv = rng.standard_normal((3 * C, C), dtype=np.float32) * C**-0.5
    w_out = rng.standard_normal((C, C), dtype=np.float32) * C**-0.5
    b_out = rng.standard_normal(C, dtype=np.float32) * 0.01
    o = kernel(x=x, w_qkv=w_qkv, w_out=w_out, b_out=b_out)
    print("kernel ran, out shape", o.shape)


# revision 38
# speedup vs baseline: 1.0219x; 1.0219x over previous
"""MHSA kernel for 8 Trainium2 NeuronCores (Bass/Tile).

Distribution: data-parallel over batch (4) x tensor-parallel over heads
(2 groups of 8 heads) = 8 shards, one per core.

Per core (batch b, head-group t), everything bf16 on the PE with fp32
PSUM accumulation:
  xT[c,n] (host-transposed)  -> qT,kT[d,n] d-major;  v[m,d] m-major with a
  ones column appended per head (so the attn@v matmul also produces the
  softmax denominator).  Softmax skips max-subtraction (scores ~ N(0,1));
  exp runs on ACT straight out of PSUM.  vhat is normalized from PSUM via
  a reciprocal + partition-broadcast DMA, and the output projection
  consumes vhatT directly, emitting the partial output in natural [n,c]
  layout.  Host sums the two TP partials per batch and adds the bias.
"""

import numpy as np
import ml_dtypes
import jax
from jax.sharding import Mesh, PartitionSpec as P

import concourse.bass as bass
import concourse.tile as tile
from concourse import mybir
from concourse.bass import ds, ts
from concourse.bass2jax import bass_jit, bass_shard_map

# ---- problem constants --------------------------------------------------
B, N, C, H = 4, 2048, 1024, 16
HD = C // H           # 64
NCORES = 8
TP = 2                # head groups
HPG = H // TP         # 8 heads per core
DPG = HPG * HD        # 512
VSTR = HD + 8         # v_aug per-head stride: 64 v dims + 1 ones + 7 pad
VAUG = HPG * VSTR     # 576
PP = 128              # partitions
SB = 512              # matmul free-dim block (one PSUM bank of fp32)

BF = mybir.dt.bfloat16
F32 = mybir.dt.float32
EXP = mybir.ActivationFunctionType.Exp
MULT = mybir.AluOpType.mult

_compiled = {}


# ---- the per-core Tile kernel ------------------------------------------
def _build_mhsa(tc, out_ap, xT, wq, wk, wv, wo, n=N, c=C, hpg=HPG, dbg=None,
                dbg2=None, exact_recip=True, ablate=(), repeat=1):
    """Per-core MHSA.  APs (no leading core dim):
      xT [c, n] bf16, wq/wk [c, dpg] bf16, wv [c, vaug] bf16,
      wo [HD, hpg, c] bf16, out [n, c] f32.
    """
    nc = tc.nc
    dpg = hpg * HD
    vaug = hpg * VSTR
    nct = c // PP          # c-tiles (contraction for projections)
    nmt = n // PP          # m-tiles (keys) == n-tiles (queries)
    ndt = dpg // PP        # d-tiles for q/k
    sbn = min(SB, n)       # free-dim block for projection outputs
    sbc = min(SB, c)
    nhalf = n // 2         # attention processes n in two halves
    sca = min(SB, nhalf)   # score sub-block
    nsc = nhalf // sca

    with tc.tile_pool(name="persist", bufs=1) as persist:
        xT_sb = persist.tile([PP, nct, n], BF)
        wq_sb = persist.tile([PP, nct, dpg], BF)
        wk_sb = persist.tile([PP, nct, dpg], BF)
        wv_sb = persist.tile([PP, nct, vaug], BF)
        wo_sb = persist.tile([PP, ndt, c], BF)
        qT_sb = persist.tile([PP, ndt, n], BF)
        kT_sb = persist.tile([PP, ndt, n], BF)
        v_sb = persist.tile([PP, nmt, vaug], BF)
        vhn_sb = persist.tile([PP, ndt, n], BF)   # unnormalized vhatT
        vhn2_sb = persist.tile([PP, ndt, n], BF)  # normalized vhatT

        xT_t = xT.rearrange("(t p) n -> t p n", p=PP)
        for c_t in range(nct):
            nc.sync.dma_start(out=xT_sb[:, c_t], in_=xT_t[c_t])
        nc.sync.dma_start(out=wq_sb, in_=wq.rearrange("(t p) d -> p t d", p=PP))
        nc.sync.dma_start(out=wk_sb, in_=wk.rearrange("(t p) d -> p t d", p=PP))
        nc.sync.dma_start(out=wv_sb, in_=wv.rearrange("(t p) d -> p t d", p=PP))
        nc.sync.dma_start(out=wo_sb, in_=wo.rearrange("(t p) c -> p t c", p=PP))

        # ---- phase 1: qT, kT = (w @ xT) in d-major [d, n] layout -------
        for _rep in range(repeat):
            _phases(tc, out_ap, xT, wq, wk, wv, wo, n, c, hpg, dbg, dbg2,
                    exact_recip, ablate, persist, xT_sb, wq_sb, wk_sb, wv_sb,
                    wo_sb, qT_sb, kT_sb, v_sb, vhn_sb, vhn2_sb)


def _phases(tc, out_ap, xT, wq, wk, wv, wo, n, c, hpg, dbg, dbg2, exact_recip,
            ablate, persist, xT_sb, wq_sb, wk_sb, wv_sb, wo_sb, qT_sb, kT_sb,
            v_sb, vhn_sb, vhn2_sb):
    nc = tc.nc
    dpg = hpg * HD
    vaug = hpg * VSTR
    nct = c // PP
    nmt = n // PP
    ndt = dpg // PP
    sbn = min(SB, n)
    sbc = min(SB, c)
    nhalf = n // 2
    sca = min(SB, nhalf)
    nsc = nhalf // sca
    if True:
        # d_t outer + q/k interleaved so attention pair 0 can start as soon
        # as v is ready; one weight tile serves all n-blocks (4 MMs per LDW)
        with tc.tile_pool(name="qk_ps", bufs=8, space=bass.MemorySpace.PSUM) as qkp:
            for d_t in range(ndt):
                for w_sb, dst in ((wk_sb, kT_sb), (wq_sb, qT_sb)):
                    pss = [qkp.tile([PP, sbn], F32, tag="qk", name="qk_ps_t")
                           for _ in range(n // sbn)]
                    for c_t in range(nct):
                        for nb in range(n // sbn):
                            nc.tensor.matmul(
                                pss[nb],
                                w_sb[:, c_t, ts(d_t, PP)],
                                xT_sb[:, c_t, ts(nb, sbn)],
                                start=(c_t == 0),
                                stop=(c_t == nct - 1),
                            )
                    for nb in range(n // sbn):
                        nc.vector.tensor_copy(dst[:, d_t, ts(nb, sbn)], pss[nb])

        # ---- phase 2: v = (x @ wv) m-major, aug layout + ones ----------
        with tc.tile_pool(name="v_ps", bufs=2, space=bass.MemorySpace.PSUM) as vp:
            for m_t in range(nmt):
                ps = vp.tile([PP, vaug], F32)
                for c_t in range(nct):
                    st, sp = (c_t == 0), (c_t == nct - 1)
                    for vb in range(0, vaug, SB):
                        ve = min(vb + SB, vaug)
                        nc.tensor.matmul(
                            ps[:, vb:ve],
                            xT_sb[:, c_t, ts(m_t, PP)],
                            wv_sb[:, c_t, vb:ve],
                            start=st,
                            stop=sp,
                        )
                nc.vector.tensor_copy(v_sb[:, m_t], ps)
                nc.vector.memset(
                    v_sb[:, m_t].rearrange("p (h w) -> p h w", w=VSTR)[:, :, HD : HD + 1],
                    1.0,
                )

        # ---- phase 3: attention, head PAIRS row-packed in the PE array --
        # Heads 2j (partitions 0:64) and 2j+1 (partitions 64:128) of kT/qT
        # tile j issue concurrent K=64 matmuls to different row groups and
        # different PSUM banks; one exp covers both heads' score block.
        with (
            tc.tile_pool(name="sc_ps", bufs=(3 if "scb3" in ablate else 2),
                         space=bass.MemorySpace.PSUM) as scp,
            tc.tile_pool(name="vh_ps", bufs=(2 if "scb3" in ablate else 4),
                         space=bass.MemorySpace.PSUM) as vhp,
            tc.tile_pool(name="e_sb", bufs=4) as ep,
            tc.tile_pool(name="bc_sb", bufs=4) as bcp,
            tc.tile_pool(name="den_sb", bufs=2) as denp,
            tc.tile_pool(name="rec_dr", bufs=2, space=bass.MemorySpace.DRAM) as drp,
        ):
            if "attn" in ablate:
                nc.vector.memset(vhn2_sb, 1.0)
            nun = 2 * (n // sca)              # den rows per head pair
            for j in range(0 if "attn" not in ablate else hpg // 2, hpg // 2):
                kA, qA = kT_sb[0:HD, j], qT_sb[0:HD, j]        # head 2j
                kB, qB = kT_sb[HD:PP, j], qT_sb[HD:PP, j]      # head 2j+1
                # den rows live at the 4 legal engine partition bases x 2
                # column halves (engine APs only address bases 0/32/64/96)
                den_p = denp.tile([97, 2 * sca], F32)
                nc.vector.memset(den_p, 1.0)
                for nb in range(n // sca):
                    vhA = vhp.tile([HD + 1, sca], F32, tag="vh")
                    vhB = vhp.tile([HD + 1, sca], F32, tag="vh")
                    for m_t in range(nmt):
                        sc = scp.tile([PP, 2 * sca], F32)
                        nc.tensor.matmul(
                            sc[:, 0:sca], kA[:, ts(m_t, PP)],
                            qA[:, ts(nb, sca)], start=True, stop=True,
                        )
                        nc.tensor.matmul(
                            sc[:, sca : 2 * sca], kB[:, ts(m_t, PP)],
                            qB[:, ts(nb, sca)], start=True, stop=True,
                        )
                        E = ep.tile([PP, 2 * sca], BF)
                        if "exp" in ablate:
                            nc.vector.tensor_copy(E, sc)
                        elif "halfexp" in ablate:
                            # timing probe only: exp half the scores
                            nc.scalar.activation(E[:, 0:sca], sc[:, 0:sca], EXP)
                        else:
                            nc.scalar.activation(E, sc, EXP)
                        st, sp = (m_t == 0), (m_t == nmt - 1)
                        eb = 0 if "halfexp" in ablate else sca
                        nc.tensor.matmul(
                            vhA, v_sb[:, m_t, ds((2 * j) * VSTR, HD + 1)],
                            E[:, 0:sca], start=st, stop=sp,
                        )
                        nc.tensor.matmul(
                            vhB, v_sb[:, m_t, ds((2 * j + 1) * VSTR, HD + 1)],
                            E[:, eb : eb + sca], start=st, stop=sp,
                        )
                    for vh, pb, u in ((vhA, 0, 2 * nb), (vhB, HD, 2 * nb + 1)):
                        # stash unnormalized vhat + its denominator row
                        nc.vector.tensor_copy(
                            vhn_sb[pb : pb + HD, j, ts(nb, sca)], vh[0:HD]
                        )
                        ub = 32 * (u % 4)
                        nc.vector.tensor_copy(
                            den_p[ub : ub + 1, ts(u // 4, sca)], vh[HD : HD + 1, :]
                        )
                if "norm" in ablate:
                    nc.vector.tensor_copy(vhn2_sb[:, j], vhn_sb[:, j])
                    continue
                # one batched reciprocal for the pair's denominators, then
                # partition-broadcast each row via a DRAM bounce
                rec_p = denp.tile([97, 2 * sca], F32, tag="rec")
                nc.vector.reciprocal(rec_p, den_p)
                dr = drp.tile([nun, sca], F32)
                for u in range(nun):
                    ub = 32 * (u % 4)
                    nc.sync.dma_start(
                        out=dr[u : u + 1],
                        in_=rec_p[ub : ub + 1, ts(u // 4, sca)],
                    )
                for nb in range(n // sca):
                    for pb, u in ((0, 2 * nb), (HD, 2 * nb + 1)):
                        bc = bcp.tile([PP, sca], F32)
                        row = dr[u : u + 1]
                        nc.sync.dma_start(
                            out=bc,
                            in_=bass.AP(
                                tensor=row.tensor, offset=row.offset,
                                ap=[[0, PP], [1, sca]],
                            ),
                        )
                        nc.vector.tensor_tensor(
                            vhn2_sb[pb : pb + HD, j, ts(nb, sca)],
                            vhn_sb[pb : pb + HD, j, ts(nb, sca)],
                            bc[pb : pb + HD], MULT,
                        )

        if dbg is not None:
            d_qT, d_kT, d_v, d_vhn = dbg
            nc.sync.dma_start(out=d_qT, in_=qT_sb)
            nc.sync.dma_start(out=d_kT, in_=kT_sb)
            nc.sync.dma_start(out=d_v, in_=v_sb)
            nc.sync.dma_start(out=d_vhn, in_=vhn2_sb)

        # ---- phase 4: out = vhatT.T @ woT in natural [n, c] layout -----
        with (
            tc.tile_pool(name="o_ps", bufs=4, space=bass.MemorySpace.PSUM) as op,
            tc.tile_pool(name="o_sb", bufs=4) as osb,
        ):
            for n_t in range(nmt):
                for cb in range(c // sbc):
                    ps = op.tile([PP, sbc], F32)
                    for d_t in range(ndt):
                        nc.tensor.matmul(
                            ps,
                            vhn2_sb[:, d_t, ts(n_t, PP)],
                            wo_sb[:, d_t, ts(cb, sbc)],
                            start=(d_t == 0),
                            stop=(d_t == ndt - 1),
                        )
                    stg = osb.tile([PP, sbc], F32)
                    nc.vector.tensor_copy(stg, ps)
                    nc.sync.dma_start(
                        out=out_ap[ts(n_t, PP), ts(cb, sbc)], in_=stg
                    )


@bass_jit
def _mhsa_core(nc, xT, wq, wk, wv, wo):
    out = nc.dram_tensor("out_part", [N, C], F32, kind="ExternalOutput")
    with tile.TileContext(nc) as tc:
        _build_mhsa(tc, out[:], xT[:], wq[:], wk[:], wv[:], wo[:])
    return (out,)


# ---- host-side sharding / driver ---------------------------------------
def _get_compiled():
    if "fn" in _compiled:
        return _compiled["fn"], _compiled["mesh"]
    devs = jax.devices()[:NCORES]
    mesh = Mesh(np.asarray(devs), ("core",))
    fn = bass_shard_map(
        _mhsa_core,
        mesh=mesh,
        in_specs=(P("core"),) * 5,
        out_specs=(P("core"),),
    )
    _compiled["fn"] = fn
    _compiled["mesh"] = mesh
    return fn, mesh


def _make_shards(x, w_qkv, w_out):
    bf = ml_dtypes.bfloat16
    wq_s = w_qkv[0:C] * np.float32(HD**-0.5)  # fold attention scale into wq
    wk_s = w_qkv[C : 2 * C]
    wv_s = w_qkv[2 * C : 3 * C]
    xT_b = [np.ascontiguousarray(x[b].T).astype(bf) for b in range(B)]
    xTs, wqs, wks, wvs, wos = [], [], [], [], []
    for cid in range(NCORES):
        b, t = cid // TP, cid % TP
        sl = slice(t * DPG, (t + 1) * DPG)
        xTs.append(xT_b[b])
        wqs.append(np.ascontiguousarray(wq_s[sl].T).astype(bf))
        wks.append(np.ascontiguousarray(wk_s[sl].T).astype(bf))
        wvT = wv_s[sl].T  # [C, DPG]
        wv_aug = np.zeros((C, VAUG), bf)
        for h in range(HPG):
            wv_aug[:, h * VSTR : h * VSTR + HD] = wvT[:, h * HD : (h + 1) * HD].astype(bf)
        wvs.append(wv_aug)
        woT = w_out[:, sl].T  # [DPG, C]
        wos.append(np.ascontiguousarray(woT).astype(bf))
    return tuple(
        np.concatenate(a, axis=0) for a in (xTs, wqs, wks, wvs, wos)
    )


def kernel(x, w_qkv, w_out, b_out):
    x = np.asarray(x, dtype=np.float32)
    w_qkv = np.asarray(w_qkv, dtype=np.float32)
    w_out = np.asarray(w_out, dtype=np.float32)
    b_out = np.asarray(b_out, dtype=np.float32)

    fn, _ = _get_compiled()
    shards = _make_shards(x, w_qkv, w_out)
    res = jax.block_until_ready(fn(*shards))
    parts = np.asarray(res[0]).reshape(NCORES, N, C)

    out = np.empty((B, N, C), dtype=np.float32)
    for b in range(B):
        out[b] = parts[2 * b] + parts[2 * b + 1] + b_out[None, :]
    return out


if __name__ == "__main__":
    rng = np.random.default_rng(0)
    x = rng.standard_normal((B, N, C), dtype=np.float32)
    w_qkv = rng.standard_normal((3 * C, C), dtype=np.float32) * C**-0.5
    w_out = rng.standard_normal((C, C), dtype=np.float32) * C**-0.5
    b_out = rng.standard_normal(C, dtype=np.float32) * 0.01
    o = kernel(x=x, w_qkv=w_qkv, w_out=w_out, b_out=b_out)
    print("kernel ran, out shape", o.shape)
